# revision 9
# baseline (speedup 1.0000x reference)
"""FLGC (soft group routing) fused 1x1 conv kernel for Trainium2, 8 cores.

Math:  s_hat = softmax(S, 1); t_hat = softmax(T, 1); mix = t_hat @ s_hat.T
       out = conv1x1(x, W * mix)   -- a 64x64 channel-mixing matmul applied
       over every (batch, h, w) position.

Strategy: data-parallel over batch B=16 -> 2 batches per core. Per core the
activations are viewed as [128, 50176] (2 batches x 64 channels stacked on
partitions). The routing math is computed on-device (replicated, tiny), the
effective weight W_effT = (W * mix)^T is placed twice on the diagonal of a
[128,128] block-diagonal stationary operand, so a single K=128 matmul
processes both batches at full PE width. Streaming is fp32 via float32r
(1 cycle/row for N>=256, exact fp32 numerics).
"""

import numpy as np
from contextlib import ExitStack

import concourse.bass as bass
import concourse.bacc as bacc
import concourse.mybir as mybir
import concourse.tile as tile
from concourse.masks import make_identity
from concourse.bass_utils import run_bass_kernel_spmd

F32 = mybir.dt.float32
F32R = mybir.dt.float32r

B, C, H, W_SP, G = 16, 64, 224, 224, 8
HWP = H * W_SP            # 50176 spatial positions per batch
NCORES = 8
BPC = B // NCORES         # 2 batches per core
P = BPC * C               # 128 partitions
CHUNK = 2048              # free-dim columns per DMA tile (1 MiB per DMA)
MM_N = 512                # moving-operand columns per matmul (1 PSUM bank fp32)


def _build_nc() -> bass.Bass:
    nc = bacc.Bacc(trn_type="TRN2", target_bir_lowering=False, debug=False,
                   num_devices=NCORES)
    x = nc.dram_tensor("x", [BPC, C, H, W_SP], F32, kind="ExternalInput")
    w = nc.dram_tensor("w", [C, C], F32, kind="ExternalInput")
    s = nc.dram_tensor("s", [C, G], F32, kind="ExternalInput")
    t = nc.dram_tensor("t", [C, G], F32, kind="ExternalInput")
    out = nc.dram_tensor("out", [BPC, C, H, W_SP], F32, kind="ExternalOutput")

    x_flat = x.ap().rearrange("b c h w -> (b c) (h w)")      # [128, 50176]
    out_flat = out.ap().rearrange("b c h w -> (b c) (h w)")  # [128, 50176]

    with tile.TileContext(nc) as tc, ExitStack() as ctx:
        const = ctx.enter_context(tc.tile_pool(name="const", bufs=1))

        # ---- routing preamble: W_effT = (W * (t_hat @ s_hat^T))^T ----
        # NOTE: walrus limits per-instruction sync waits (ACT: 1 slot).
        # Every instruction below is arranged to carry at most 1 cross-
        # engine wait (<=2 for PE/DVE/DMA), funneling DMA deps through DVE.
        with tc.tile_pool(name="psum_pre", bufs=1, space="PSUM") as psum_pre:
            ident = const.tile([C, C], F32)
            make_identity(nc, ident)
            # absorb gpsimd (identity) into PE's clock before real work
            dummy0 = psum_pre.tile([C, C], F32)
            nc.tensor.transpose(dummy0, ident, ident)

            st = const.tile([C, 2 * G], F32)        # S | T side by side
            nc.sync.dma_start(st[:, 0:G], s.ap())
            nc.sync.dma_start(st[:, G:2 * G], t.ap())
            # softmax along G: max first (funnels DMA deps through DVE so
            # the ACT exp carries a single DVE wait; also matches jax).
            mx = const.tile([C, 2], F32)
            nc.vector.reduce_max(mx[:, 0:1], st[:, 0:G], axis=mybir.AxisListType.X)
            nc.vector.reduce_max(mx[:, 1:2], st[:, G:2 * G], axis=mybir.AxisListType.X)
            nmx = const.tile([C, 2], F32)
            nc.vector.tensor_scalar_mul(nmx, mx, -1.0)
            # stage st through DVE so the ACT exps' data AND bias are both
            # DVE-produced -> a single merged wait (ACT has 1 wait slot).
            stx = const.tile([C, 2 * G], F32)
            nc.vector.tensor_copy(stx, st)
            nc.scalar.activation(
                stx[:, 0:G], stx[:, 0:G], mybir.ActivationFunctionType.Exp,
                bias=nmx[:, 0:1],
            )
            nc.scalar.activation(
                stx[:, G:2 * G], stx[:, G:2 * G], mybir.ActivationFunctionType.Exp,
                bias=nmx[:, 1:2],
            )
            sums = const.tile([C, 2], F32)
            nc.vector.reduce_sum(sums[:, 0:1], stx[:, 0:G], axis=mybir.AxisListType.X)
            nc.vector.reduce_sum(sums[:, 1:2], stx[:, G:2 * G], axis=mybir.AxisListType.X)
            recips = const.tile([C, 2], F32)
            nc.vector.reciprocal(recips, sums)
            nc.vector.tensor_scalar_mul(stx[:, 0:G], stx[:, 0:G], recips[:, 0:1])
            nc.vector.tensor_scalar_mul(stx[:, G:2 * G], stx[:, G:2 * G], recips[:, 1:2])

            # transpose s_hat, t_hat to [G, C]
            pt_s = psum_pre.tile([G, C], F32)
            nc.tensor.transpose(pt_s, stx[:, 0:G], ident)
            pt_t = psum_pre.tile([G, C], F32)
            nc.tensor.transpose(pt_t, stx[:, G:2 * G], ident)
            sT = const.tile([G, C], F32)
            tT = const.tile([G, C], F32)
            nc.vector.tensor_copy(sT, pt_s)
            nc.vector.tensor_copy(tT, pt_t)

            # mixT[c, o] = sum_g s_hat[c, g] * t_hat[o, g]
            pmix = psum_pre.tile([C, C], F32)
            nc.tensor.matmul(pmix, lhsT=sT, rhs=tT, start=True, stop=True)
            mixS = const.tile([C, C], F32)
            nc.vector.tensor_copy(mixS, pmix)

            # W^T then elementwise W_effT = W^T * mixT
            w_sb = const.tile([C, C], F32)
            nc.sync.dma_start(w_sb, w.ap())
            pwT = psum_pre.tile([C, C], F32)
            nc.tensor.transpose(pwT, w_sb, ident)
            wTs = const.tile([C, C], F32)
            nc.vector.tensor_copy(wTs, pwT)
            weffT = const.tile([C, C], F32)
            nc.vector.tensor_mul(weffT, mixS, wTs)

            # block-diagonal stationary operand [128, 128]; memset on DVE so
            # the two block DMAs wait on a single engine clock.
            bd = const.tile([P, P], F32)
            nc.vector.memset(bd, 0.0)
            nc.sync.dma_start(bd[0:C, 0:C], weffT)
            nc.sync.dma_start(bd[C:P, C:P], weffT)
            # absorb the bd DMA-lane deps into DVE's clock so the first main
            # matmul carries only its own input-DMA wait plus one DVE wait.
            trash = const.tile([P, C], F32)
            nc.vector.tensor_copy(trash[0:C, :], bd[0:C, 0:C])
            nc.vector.tensor_copy(trash[C:P, :], bd[C:P, C:P])

        # ---- main loop: stream x through the PE ----
        inp = ctx.enter_context(tc.tile_pool(name="inp", bufs=4))
        outp = ctx.enter_context(tc.tile_pool(name="outp", bufs=4))
        psum = ctx.enter_context(tc.tile_pool(name="psum", bufs=8, space="PSUM"))

        offs = [(i * CHUNK, CHUNK) for i in range(HWP // CHUNK)]
        if HWP % CHUNK:
            offs.append(((HWP // CHUNK) * CHUNK, HWP % CHUNK))

        for off, F in offs:
            xin = inp.tile([P, CHUNK], F32, tag="xin")
            nc.sync.dma_start(xin[:, 0:F], x_flat[:, off:off + F])
            yout = outp.tile([P, CHUNK], F32, tag="yout")
            for j in range(F // MM_N):
                pm = psum.tile([P, MM_N], F32, tag="pm")
                nc.tensor.matmul(
                    pm,
                    lhsT=bd,
                    rhs=xin[:, j * MM_N:(j + 1) * MM_N],
                    start=True,
                    stop=True,
                )
                nc.vector.tensor_copy(yout[:, j * MM_N:(j + 1) * MM_N], pm)
            nc.sync.dma_start(out_flat[:, off:off + F], yout[:, 0:F])

    nc.compile()
    return nc


_CACHE = {}


def _get_nc() -> bass.Bass:
    if "nc" not in _CACHE:
        _CACHE["nc"] = _build_nc()
    return _CACHE["nc"]


def run(inputs, trace=False, **kw):
    x = np.ascontiguousarray(np.asarray(inputs["x"], dtype=np.float32))
    W = np.ascontiguousarray(np.asarray(inputs["W"], dtype=np.float32).reshape(C, C))
    S = np.ascontiguousarray(np.asarray(inputs["S"], dtype=np.float32))
    T = np.ascontiguousarray(np.asarray(inputs["T"], dtype=np.float32))
    in_maps = [
        {"x": x[c * BPC:(c + 1) * BPC], "w": W, "s": S, "t": T}
        for c in range(NCORES)
    ]
    nc = _get_nc()
    res = run_bass_kernel_spmd(nc, in_maps, list(range(NCORES)), trace=trace, **kw)
    out = np.concatenate([res.results[c]["out"] for c in range(NCORES)], axis=0)
    return out, res


def kernel(**inputs) -> np.ndarray:
    return run(inputs)[0]


# revision 11
# speedup vs baseline: 1.0269x; 1.0269x over previous
"""FLGC (soft group routing) fused 1x1 conv kernel for Trainium2, 8 cores.

Math:  s_hat = softmax(S, 1); t_hat = softmax(T, 1); mix = t_hat @ s_hat.T
       out = conv1x1(x, W * mix)   -- a 64x64 channel-mixing matmul applied
       over every (batch, h, w) position.

Strategy: data-parallel over batch B=16 -> 2 batches per core. Per core the
activations are viewed as [128, 50176] (2 batches x 64 channels stacked on
partitions). The routing math is computed on-device (replicated, tiny), the
effective weight W_effT = (W * mix)^T is placed twice on the diagonal of a
[128,128] block-diagonal stationary operand, so a single K=128 matmul
processes both batches at full PE width. Streaming is fp32 via float32r
(1 cycle/row for N>=256, exact fp32 numerics).
"""

import numpy as np
from contextlib import ExitStack

import concourse.bass as bass
import concourse.bacc as bacc
import concourse.mybir as mybir
import concourse.tile as tile
from concourse.masks import make_identity
from concourse.bass_utils import run_bass_kernel_spmd

F32 = mybir.dt.float32
F32R = mybir.dt.float32r

B, C, H, W_SP, G = 16, 64, 224, 224, 8
HWP = H * W_SP            # 50176 spatial positions per batch
NCORES = 8
BPC = B // NCORES         # 2 batches per core
P = BPC * C               # 128 partitions
CHUNK = 2048              # free-dim columns per DMA tile (1 MiB per DMA)
MM_N = 512                # moving-operand columns per matmul (1 PSUM bank fp32)


def _build_nc() -> bass.Bass:
    nc = bacc.Bacc(trn_type="TRN2", target_bir_lowering=False, debug=False,
                   num_devices=NCORES)
    x = nc.dram_tensor("x", [BPC, C, H, W_SP], F32, kind="ExternalInput")
    w = nc.dram_tensor("w", [C, C], F32, kind="ExternalInput")
    s = nc.dram_tensor("s", [C, G], F32, kind="ExternalInput")
    t = nc.dram_tensor("t", [C, G], F32, kind="ExternalInput")
    out = nc.dram_tensor("out", [BPC, C, H, W_SP], F32, kind="ExternalOutput")

    x_flat = x.ap().rearrange("b c h w -> (b c) (h w)")      # [128, 50176]
    out_flat = out.ap().rearrange("b c h w -> (b c) (h w)")  # [128, 50176]

    with tile.TileContext(nc) as tc, ExitStack() as ctx:
        const = ctx.enter_context(tc.tile_pool(name="const", bufs=1))

        # main-loop pools up front so the first input DMAs can be emitted
        # (and issued) before the routing preamble occupies the SP ring.
        inp = ctx.enter_context(tc.tile_pool(name="inp", bufs=6))
        outp = ctx.enter_context(tc.tile_pool(name="outp", bufs=6))

        offs = [(i * CHUNK, CHUNK) for i in range(HWP // CHUNK)]
        if HWP % CHUNK:
            offs.append(((HWP // CHUNK) * CHUNK, HWP % CHUNK))

        xins = []
        for off, F in offs[:2]:
            xin = inp.tile([P, CHUNK], F32, tag="xin")
            nc.sync.dma_start(xin[:, 0:F], x_flat[:, off:off + F])
            xins.append(xin)

        # ---- routing preamble: W_effT = (W * (t_hat @ s_hat^T))^T ----
        # NOTE: walrus limits per-instruction sync waits (ACT: 1 slot).
        # Every instruction below is arranged to carry at most 1 cross-
        # engine wait (<=2 for PE/DVE/DMA), funneling DMA deps through DVE.
        with tc.tile_pool(name="psum_pre", bufs=1, space="PSUM") as psum_pre:
            ident = const.tile([C, C], F32)
            make_identity(nc, ident)
            # absorb gpsimd (identity) into PE's clock before real work
            dummy0 = psum_pre.tile([C, C], F32)
            nc.tensor.transpose(dummy0, ident, ident)

            st = const.tile([C, 2 * G], F32)        # S | T side by side
            nc.sync.dma_start(st[:, 0:G], s.ap())
            nc.sync.dma_start(st[:, G:2 * G], t.ap())
            # softmax along G: max first (funnels DMA deps through DVE so
            # the ACT exp carries a single DVE wait; also matches jax).
            mx = const.tile([C, 2], F32)
            nc.vector.reduce_max(mx[:, 0:1], st[:, 0:G], axis=mybir.AxisListType.X)
            nc.vector.reduce_max(mx[:, 1:2], st[:, G:2 * G], axis=mybir.AxisListType.X)
            nmx = const.tile([C, 2], F32)
            nc.vector.tensor_scalar_mul(nmx, mx, -1.0)
            # stage st through DVE so the ACT exps' data AND bias are both
            # DVE-produced -> a single merged wait (ACT has 1 wait slot).
            stx = const.tile([C, 2 * G], F32)
            nc.vector.tensor_copy(stx, st)
            nc.scalar.activation(
                stx[:, 0:G], stx[:, 0:G], mybir.ActivationFunctionType.Exp,
                bias=nmx[:, 0:1],
            )
            nc.scalar.activation(
                stx[:, G:2 * G], stx[:, G:2 * G], mybir.ActivationFunctionType.Exp,
                bias=nmx[:, 1:2],
            )
            sums = const.tile([C, 2], F32)
            nc.vector.reduce_sum(sums[:, 0:1], stx[:, 0:G], axis=mybir.AxisListType.X)
            nc.vector.reduce_sum(sums[:, 1:2], stx[:, G:2 * G], axis=mybir.AxisListType.X)
            recips = const.tile([C, 2], F32)
            nc.vector.reciprocal(recips, sums)
            nc.vector.tensor_scalar_mul(stx[:, 0:G], stx[:, 0:G], recips[:, 0:1])
            nc.vector.tensor_scalar_mul(stx[:, G:2 * G], stx[:, G:2 * G], recips[:, 1:2])

            # transpose s_hat, t_hat to [G, C]
            pt_s = psum_pre.tile([G, C], F32)
            nc.tensor.transpose(pt_s, stx[:, 0:G], ident)
            pt_t = psum_pre.tile([G, C], F32)
            nc.tensor.transpose(pt_t, stx[:, G:2 * G], ident)
            sT = const.tile([G, C], F32)
            tT = const.tile([G, C], F32)
            nc.vector.tensor_copy(sT, pt_s)
            nc.vector.tensor_copy(tT, pt_t)

            # mixT[c, o] = sum_g s_hat[c, g] * t_hat[o, g]
            pmix = psum_pre.tile([C, C], F32)
            nc.tensor.matmul(pmix, lhsT=sT, rhs=tT, start=True, stop=True)
            mixS = const.tile([C, C], F32)
            nc.vector.tensor_copy(mixS, pmix)

            # W^T then elementwise W_effT = W^T * mixT
            w_sb = const.tile([C, C], F32)
            nc.sync.dma_start(w_sb, w.ap())
            pwT = psum_pre.tile([C, C], F32)
            nc.tensor.transpose(pwT, w_sb, ident)
            wTs = const.tile([C, C], F32)
            nc.vector.tensor_copy(wTs, pwT)
            weffT = const.tile([C, C], F32)
            nc.vector.tensor_mul(weffT, mixS, wTs)

            # block-diagonal stationary operand [128, 128]; memset on DVE so
            # the two block DMAs wait on a single engine clock.
            bd = const.tile([P, P], F32)
            nc.vector.memset(bd, 0.0)
            nc.sync.dma_start(bd[0:C, 0:C], weffT)
            nc.sync.dma_start(bd[C:P, C:P], weffT)
            # absorb the bd DMA-lane deps into DVE's clock so the first main
            # matmul carries only its own input-DMA wait plus one DVE wait.
            trash = const.tile([P, C], F32)
            nc.vector.tensor_copy(trash[0:C, :], bd[0:C, 0:C])
            nc.vector.tensor_copy(trash[C:P, :], bd[C:P, C:P])

        # ---- main loop: stream x through the PE ----
        # input DMAs ride the SP HWDGE ring; output DMAs the ACT HWDGE ring.
        psum = ctx.enter_context(tc.tile_pool(name="psum", bufs=8, space="PSUM"))

        for idx, (off, F) in enumerate(offs):
            if idx < len(xins):
                xin = xins[idx]
            else:
                xin = inp.tile([P, CHUNK], F32, tag="xin")
                nc.sync.dma_start(xin[:, 0:F], x_flat[:, off:off + F])
            yout = outp.tile([P, CHUNK], F32, tag="yout")
            for j in range(F // MM_N):
                pm = psum.tile([P, MM_N], F32, tag="pm")
                nc.tensor.matmul(
                    pm,
                    lhsT=bd,
                    rhs=xin[:, j * MM_N:(j + 1) * MM_N],
                    start=True,
                    stop=True,
                )
                nc.vector.tensor_copy(yout[:, j * MM_N:(j + 1) * MM_N], pm)
            nc.scalar.dma_start(out_flat[:, off:off + F], yout[:, 0:F])

    nc.compile()
    return nc


_CACHE = {}


def _get_nc() -> bass.Bass:
    if "nc" not in _CACHE:
        _CACHE["nc"] = _build_nc()
    return _CACHE["nc"]


def run(inputs, trace=False, **kw):
    x = np.ascontiguousarray(np.asarray(inputs["x"], dtype=np.float32))
    W = np.ascontiguousarray(np.asarray(inputs["W"], dtype=np.float32).reshape(C, C))
    S = np.ascontiguousarray(np.asarray(inputs["S"], dtype=np.float32))
    T = np.ascontiguousarray(np.asarray(inputs["T"], dtype=np.float32))
    in_maps = [
        {"x": x[c * BPC:(c + 1) * BPC], "w": W, "s": S, "t": T}
        for c in range(NCORES)
    ]
    nc = _get_nc()
    res = run_bass_kernel_spmd(nc, in_maps, list(range(NCORES)), trace=trace, **kw)
    out = np.concatenate([res.results[c]["out"] for c in range(NCORES)], axis=0)
    return out, res


def kernel(**inputs) -> np.ndarray:
    return run(inputs)[0]
